# revision 5
# baseline (speedup 1.0000x reference)
"""Multi-head attention (H=16, F=1024, b=4, t=1024) on 8 Trainium2 cores.

Sharding: core c = bi*2 + hg  (bi = batch 0..3, hg = head-group 0..1).
Each core handles 8 heads x 1 batch:
  - projects q,k,v for its 512 feature-columns (8 heads x 64)
  - computes scores twice (normal [t1,t2] layout for the attn output and
    softmax, transposed [t2,t1] layout to feed attn @ v without
    transposing the attention matrix)
  - softmax denominators come from a ones-column appended to v
    (row 64 of the xT accumulator)
  - final linear: out_partial = x @ Wf[x-rows] + q @ Wf[q-rows] with the
    core's 512+512 rows of Wf; host sums the two partials per batch and
    adds bias + residual.
All matmul operands are float32r (full PE rate, ~2e-4 rounding).
"""
import numpy as np

from concourse import mybir, tile, bacc
from concourse.bass_utils import run_bass_kernel_spmd
from concourse.masks import make_identity

F32 = mybir.dt.float32
F32R = mybir.dt.float32r
U8 = mybir.dt.uint8
AF = mybir.ActivationFunctionType
ALU = mybir.AluOpType

H, F, D = 16, 1024, 64
B, T = 4, 1024
NCORES = 8
HPC = 8          # heads per core
KT = 8           # 128-row k-tiles in F
TT = 8           # 128-row tiles in T
NEG = -1e30
EPS = 1e-30
SCALE = 1.0 / np.sqrt(D)


def _build_nc():
    nc = bacc.Bacc(None, target_bir_lowering=False, debug=False)

    # ---- DRAM I/O (per-core contents supplied by the host) ----
    qTb = nc.dram_tensor("qTb", [F, T], F32R, kind="ExternalInput")      # query[bi].T
    kTb = nc.dram_tensor("kTb", [F, T], F32R, kind="ExternalInput")      # key[bi].T
    vTb = nc.dram_tensor("vTb", [F, T], F32R, kind="ExternalInput")      # value[bi].T
    wq = nc.dram_tensor("wq", [F, 512], F32R, kind="ExternalInput")      # Wq[:, hg cols]
    wk = nc.dram_tensor("wk", [F, 512], F32R, kind="ExternalInput")
    wv = nc.dram_tensor("wv", [F, 512], F32R, kind="ExternalInput")
    wfx = nc.dram_tensor("wfx", [512, F], F32R, kind="ExternalInput")    # Wf x-rows
    wfq = nc.dram_tensor("wfq", [512, F], F32R, kind="ExternalInput")    # Wf q-rows
    qTh = nc.dram_tensor("qTh", [512, T], F32R, kind="ExternalInput")    # queryT hg rows
    mq = nc.dram_tensor("mq", [HPC, T, T], U8, kind="ExternalInput")     # 1-mask [t1,t2]
    kt_ = nc.dram_tensor("kt", [HPC, T, T], U8, kind="ExternalInput")    # mask.T|~qm [t2,t1]
    qmc = nc.dram_tensor("qmc", [128, TT], F32, kind="ExternalInput")    # qm col-tiles
    bqc = nc.dram_tensor("bqc", [128, 4], F32, kind="ExternalInput")     # bq row-tiles
    bkc = nc.dram_tensor("bkc", [128, 4], F32, kind="ExternalInput")

    attn_o = nc.dram_tensor("attn_o", [HPC, T, T], F32, kind="ExternalOutput")
    out_o = nc.dram_tensor("out_o", [T, F], F32, kind="ExternalOutput")

    with tile.TileContext(nc) as tc:
        with (
            tc.tile_pool(name="persist", bufs=1) as pp,
            tc.tile_pool(name="consts", bufs=1) as cp,
        ):
            # persistent SBUF
            qT = [pp.tile([128, T], F32R, name=f"qT{r}") for r in range(4)]
            kT = [pp.tile([128, T], F32R, name=f"kT{r}") for r in range(4)]
            vS = [pp.tile([128, HPC * 65], F32R, name=f"vS{t}") for t in range(TT)]
            xT = [pp.tile([128, T], F32R, name=f"xT{r}") for r in range(4)]
            qmc_sb = cp.tile([128, TT], F32)
            bqc_sb = cp.tile([128, 4], F32)
            bkc_sb = cp.tile([128, 4], F32)
            ident = cp.tile([128, 128], F32)
            make_identity(nc, ident)
            eps_c = cp.tile([1, 1], F32)
            nc.vector.memset(eps_c[:], EPS)
            nc.sync.dma_start(qmc_sb[:], qmc[:])
            nc.sync.dma_start(bqc_sb[:], bqc[:])
            nc.sync.dma_start(bkc_sb[:], bkc[:])

            # ---------------- Phase A: projections ----------------
            with (
                tc.tile_pool(name="ldA", bufs=1) as ldA,
                tc.tile_pool(name="wA", bufs=1) as wA,
                tc.tile_pool(name="psA", bufs=2, space="PSUM") as psA,
            ):
                # q then k: stationary = weight tiles, moving = xT chunks
                for which, (src, wdram, dst, bias) in enumerate([
                    (qTb, wq, qT, bqc_sb), (kTb, wk, kT, bkc_sb)]):
                    act = ldA.tile([128, KT * T], F32R, tag="act")
                    w_sb = wA.tile([128, KT * 512], F32R, tag="w")
                    nc.sync.dma_start(
                        act[:].rearrange("p (k t) -> p k t", k=KT),
                        src.rearrange("(k p) t -> p k t", p=128))
                    nc.sync.dma_start(
                        w_sb[:].rearrange("p (k c) -> p k c", k=KT),
                        wdram.rearrange("(k p) c -> p k c", p=128))
                    for rt in range(4):
                        for ch in range(2):
                            ps = psA.tile([128, 512], F32, tag="ps")
                            for k in range(KT):
                                nc.tensor.matmul(
                                    ps[:],
                                    w_sb[:, k * 512 + rt * 128:k * 512 + (rt + 1) * 128],
                                    act[:, k * T + ch * 512:k * T + ch * 512 + 512],
                                    start=(k == 0), stop=(k == KT - 1))
                            nc.scalar.activation(
                                dst[rt][:, ch * 512:(ch + 1) * 512], ps[:],
                                AF.Identity, bias=bias[:, rt:rt + 1])
                # v: stationary = valueT tiles, moving = wv chunks
                act = ldA.tile([128, KT * T], F32R, tag="act")
                w_sb = wA.tile([128, KT * 512], F32R, tag="w")
                nc.sync.dma_start(
                    act[:].rearrange("p (k t) -> p k t", k=KT),
                    vTb.rearrange("(k p) t -> p k t", p=128))
                nc.sync.dma_start(
                    w_sb[:].rearrange("p (k c) -> p k c", k=KT),
                    wv.rearrange("(k p) c -> p k c", p=128))
                for t2t in range(TT):
                    ps = psA.tile([128, 512], F32, tag="ps")
                    for k in range(KT):
                        nc.tensor.matmul(
                            ps[:],
                            act[:, k * T + t2t * 128:k * T + (t2t + 1) * 128],
                            w_sb[:, k * 512:(k + 1) * 512],
                            start=(k == 0), stop=(k == KT - 1))
                    nc.any.tensor_copy(
                        vS[t2t][:].rearrange("p (h c) -> p h c", h=HPC)[:, :, 0:64],
                        ps[:].rearrange("p (h c) -> p h c", h=HPC))
                    nc.vector.memset(
                        vS[t2t][:].rearrange("p (h c) -> p h c", h=HPC)[:, :, 64:65]
                        .bitcast(F32), 1.0)

            # ---------------- Phase B: attention per head ----------------
            with (
                tc.tile_pool(name="mload", bufs=2) as mload,
                tc.tile_pool(name="ew", bufs=3) as ew,
                tc.tile_pool(name="hd", bufs=2) as hd,
                tc.tile_pool(name="psS", bufs=2, space="PSUM") as psS,
                tc.tile_pool(name="psX", bufs=1, space="PSUM") as psX,
                tc.tile_pool(name="psT", bufs=2, space="PSUM") as psT,
            ):
                for j in range(HPC):
                    rt, ro = j // 2, (j % 2) * 64
                    qTs = qT[rt][ro:ro + 64, :]
                    kTs = kT[rt][ro:ro + 64, :]

                    # ---- T side: scoresT -> exp -> xT (+ ones row = denom) ----
                    ktm = mload.tile([128, TT * T], U8, tag="ktm", name=f"ktm{j}")
                    nc.sync.dma_start(
                        ktm[:].rearrange("p (a t) -> p a t", a=TT),
                        kt_[j].rearrange("(a p) t -> p a t", p=128))
                    ps_x = psX.tile([128, T], F32, tag="psx", name=f"psx{j}")
                    for t2t in range(TT):
                        ps_sT = psS.tile([128, T], F32, tag="s", name=f"psT{j}_{t2t}")
                        for ch in range(2):
                            nc.tensor.matmul(
                                ps_sT[:, ch * 512:(ch + 1) * 512],
                                kTs[:, t2t * 128:(t2t + 1) * 128],
                                qTs[:, ch * 512:ch * 512 + 512],
                                start=True, stop=True)
                        smT = ew.tile([128, T], F32, tag="smT", name=f"smT{j}_{t2t}")
                        nc.vector.scalar_tensor_tensor(
                            smT[:], ktm[:, t2t * T:(t2t + 1) * T], NEG, ps_sT[:],
                            op0=ALU.mult, op1=ALU.add)
                        eT = ew.tile([128, T], F32R, tag="eT", name=f"eT{j}_{t2t}")
                        nc.scalar.activation(eT[:], smT[:], AF.Exp, scale=SCALE)
                        for ch in range(2):
                            nc.tensor.matmul(
                                ps_x[0:65, ch * 512:(ch + 1) * 512],
                                vS[t2t][:, j * 65:(j + 1) * 65],
                                eT[:, ch * 512:(ch + 1) * 512],
                                start=(t2t == 0), stop=(t2t == TT - 1))

                    # ---- head tail: denominators + xT eviction ----
                    srow = hd.tile([1, T], F32, tag="srow", name=f"srow{j}")
                    nc.scalar.activation(srow[:], ps_x[64:65, :], AF.Identity, bias=eps_c[0:1, 0:1])
                    rrow = hd.tile([1, T], F32, tag="rrow", name=f"rrow{j}")
                    nc.vector.reciprocal(rrow[:], srow[:])
                    rbc = hd.tile([64, T], F32, tag="rbc", name=f"rbc{j}")
                    nc.gpsimd.partition_broadcast(rbc[:], rrow[:], channels=64)
                    nc.vector.tensor_tensor(
                        xT[rt][ro:ro + 64, :], ps_x[0:64, :], rbc[:], op=ALU.mult)
                    ps_s = psT.tile([128, TT], F32, tag="pss", name=f"pss{j}")
                    for i in range(TT):
                        nc.tensor.transpose(
                            ps_s[:, i:i + 1], rrow[:, i * 128:(i + 1) * 128],
                            ident[0:1, 0:1])
                    scol = hd.tile([128, TT], F32, tag="scol", name=f"scol{j}")
                    nc.vector.tensor_tensor(scol[:], qmc_sb[:], ps_s[:], op=ALU.mult)

                    # ---- N side: scores -> exp -> (exp*s)*mcomp -> DRAM ----
                    mqm = mload.tile([128, TT * T], U8, tag="mqm", name=f"mqm{j}")
                    nc.sync.dma_start(
                        mqm[:].rearrange("p (a t) -> p a t", a=TT),
                        mq[j].rearrange("(a p) t -> p a t", p=128))
                    for i in range(TT):
                        ps_sN = psS.tile([128, T], F32, tag="s", name=f"psN{j}_{i}")
                        for ch in range(2):
                            nc.tensor.matmul(
                                ps_sN[:, ch * 512:(ch + 1) * 512],
                                qTs[:, i * 128:(i + 1) * 128],
                                kTs[:, ch * 512:ch * 512 + 512],
                                start=True, stop=True)
                        eN = ew.tile([128, T], F32, tag="eN", name=f"eN{j}_{i}")
                        nc.scalar.activation(eN[:], ps_sN[:], AF.Exp, scale=SCALE)
                        af = ew.tile([128, T], F32, tag="af", name=f"af{j}_{i}")
                        nc.vector.scalar_tensor_tensor(
                            af[:], eN[:], scol[:, i:i + 1], mqm[:, i * T:(i + 1) * T],
                            op0=ALU.mult, op1=ALU.mult)
                        nc.sync.dma_start(attn_o[j, i * 128:(i + 1) * 128, :], af[:])

            # ---------------- Phase D: final linear partial ----------------
            with (
                tc.tile_pool(name="ldD", bufs=1) as ldD,
                tc.tile_pool(name="oD", bufs=2) as oD,
                tc.tile_pool(name="psD", bufs=2, space="PSUM") as psD,
            ):
                wfx_sb = ldD.tile([128, 4 * F], F32R)
                wfq_sb = ldD.tile([128, 4 * F], F32R)
                qTh_sb = ldD.tile([128, 4 * T], F32R)
                nc.sync.dma_start(
                    wfx_sb[:].rearrange("p (k c) -> p k c", k=4),
                    wfx.rearrange("(k p) c -> p k c", p=128))
                nc.sync.dma_start(
                    wfq_sb[:].rearrange("p (k c) -> p k c", k=4),
                    wfq.rearrange("(k p) c -> p k c", p=128))
                nc.sync.dma_start(
                    qTh_sb[:].rearrange("p (k t) -> p k t", k=4),
                    qTh.rearrange("(k p) t -> p k t", p=128))
                for tt in range(TT):
                    osb = oD.tile([128, F], F32, tag="osb", name=f"osb{tt}")
                    for oc in range(2):
                        ps = psD.tile([128, 512], F32, tag="ps", name=f"psD{tt}_{oc}")
                        for k in range(4):
                            nc.tensor.matmul(
                                ps[:], xT[k][:, tt * 128:(tt + 1) * 128],
                                wfx_sb[:, k * F + oc * 512:k * F + oc * 512 + 512],
                                start=(k == 0), stop=False)
                        for k in range(4):
                            nc.tensor.matmul(
                                ps[:], qTh_sb[:, k * T + tt * 128:k * T + (tt + 1) * 128],
                                wfq_sb[:, k * F + oc * 512:k * F + oc * 512 + 512],
                                start=False, stop=(k == 3))
                        nc.any.tensor_copy(osb[:, oc * 512:(oc + 1) * 512], ps[:])
                    nc.sync.dma_start(out_o[tt * 128:(tt + 1) * 128, :], osb[:])

    nc.compile()
    return nc


_NC = None


def _get_nc():
    global _NC
    if _NC is None:
        _NC = _build_nc()
    return _NC


def _prep_inputs(query, key, value, mask, query_mask, Wq, Wk, Wv, Wf, bq, bk):
    """Build the 8 per-core input maps from the full inputs."""
    f32 = np.float32
    qT = [np.ascontiguousarray(query[b].T, dtype=f32) for b in range(B)]
    kT = [np.ascontiguousarray(key[b].T, dtype=f32) for b in range(B)]
    vT = [np.ascontiguousarray(value[b].T, dtype=f32) for b in range(B)]
    mb = np.asarray(mask, dtype=bool)
    qm = np.asarray(query_mask, dtype=f32)
    qmb = qm != 0.0

    in_maps = []
    for c in range(NCORES):
        bi, hg = c // 2, c % 2
        cols = slice(512 * hg, 512 * (hg + 1))
        heads = [8 * hg + j for j in range(HPC)]
        idx = [h * B + bi for h in heads]
        msel = mb[idx]                                    # [8, t1, t2]
        mq_c = (~msel).astype(np.uint8)
        kt_c = (np.transpose(msel, (0, 2, 1)) | ~qmb[bi][None, None, :]).astype(np.uint8)
        in_maps.append({
            "qTb": qT[bi], "kTb": kT[bi], "vTb": vT[bi],
            "wq": np.ascontiguousarray(Wq[:, cols], dtype=f32),
            "wk": np.ascontiguousarray(Wk[:, cols], dtype=f32),
            "wv": np.ascontiguousarray(Wv[:, cols], dtype=f32),
            "wfx": np.ascontiguousarray(Wf[cols, :], dtype=f32),
            "wfq": np.ascontiguousarray(Wf[F + 512 * hg:F + 512 * (hg + 1), :], dtype=f32),
            "qTh": np.ascontiguousarray(qT[bi][cols, :]),
            "mq": mq_c, "kt": kt_c,
            "qmc": np.ascontiguousarray(qm[bi].reshape(TT, 128).T, dtype=f32),
            "bqc": np.ascontiguousarray(
                np.asarray(bq, f32)[512 * hg:512 * (hg + 1)].reshape(4, 128).T),
            "bkc": np.ascontiguousarray(
                np.asarray(bk, f32)[512 * hg:512 * (hg + 1)].reshape(4, 128).T),
        })
    return in_maps


def kernel(query, key, value, mask, query_mask, Wq, bq, Wk, bk, Wv, bv, Wf, bf,
           _trace=False):
    query = np.asarray(query, np.float32)
    key = np.asarray(key, np.float32)
    value = np.asarray(value, np.float32)
    Wq = np.asarray(Wq, np.float32)
    Wk = np.asarray(Wk, np.float32)
    Wv = np.asarray(Wv, np.float32)
    Wf = np.asarray(Wf, np.float32)
    bf = np.asarray(bf, np.float32)
    bv = np.asarray(bv, np.float32)
    qm = np.asarray(query_mask, np.float32)

    nc = _get_nc()
    in_maps = _prep_inputs(query, key, value, mask, query_mask, Wq, Wk, Wv, Wf, bq, bk)
    res = run_bass_kernel_spmd(nc, in_maps, core_ids=list(range(NCORES)),
                               trace=_trace)

    attn = np.empty((H * B, T, T), np.float32)
    out = np.empty((B, T, F), np.float32)
    for bi in range(B):
        p = res.results[bi * 2]["out_o"] + res.results[bi * 2 + 1]["out_o"]
        out[bi] = p + bf[None, :] + query[bi]
        for hg in range(2):
            c = bi * 2 + hg
            for j in range(HPC):
                attn[(8 * hg + j) * B + bi] = res.results[c]["attn_o"][j]
    if np.any(bv != 0.0):
        # x rows sum to qm[t] post-softmax, so bv contributes qm ⊗ bv @ Wf[:F].
        corr = np.einsum("bt,f,fo->bto", qm, bv, Wf[:F].astype(np.float64),
                         optimize=True).astype(np.float32)
        out += corr
    if _trace:
        kernel._last_results = res
    return out, attn
